# revision 26
# baseline (speedup 1.0000x reference)
"""Deformable self-attention kernel for Trainium2 (8 NeuronCores).

Structural reduction: the sampling offsets are ``tanh(...) * (2/128)`` with
``|tanh| < 1``, added to *integer* grid coordinates and then rounded.  Since
the perturbation magnitude is < 0.5, ``round(c + d) == c`` always, so the
gather indices are exactly ``arange(N)`` (identity), independent of the data.
Each token attends only to itself at all 7 points; the 7 scores are equal, so
softmax is uniform and the attention output equals ``v``.  The whole module
therefore computes

    out = (x @ Wv + bv) @ Wo + bo = x @ (Wv @ Wo) + (bv @ Wo + bo)

Device strategy (per sharding_hint, row-parallel over the N axis):
  - each core gets 2048 tokens of x, fed pre-transposed ([D, T] layout) and
    pre-rounded to the fp32r grid (fp32 with 12-bit mantissa, the PE's fast
    fp32 streaming mode) — layout/dtype marshaling done while sharding;
  - Wv is fed transposed so the on-device fold W = Wv @ Wo (full fp32) needs
    no PE transposes; the PSUM->SBUF copy rounds W to fp32r;
  - the main [2048, 512] @ [512, 512] matmul runs in fp32r at 1 cycle/row;
  - outputs are copied PSUM->SBUF alternating DVE/ACT and stored with 1 MB
    batched DMAs.
"""

import os
import sys

import numpy as np

for _p in ("/opt/trn_rl_repo", "/root/.axon_site/_ro/trn_rl_repo"):
    if os.path.isdir(_p) and _p not in sys.path:
        sys.path.append(_p)

import concourse.bass as bass
import concourse.mybir as mybir
import concourse.tile as tile
from concourse import bacc
from concourse.bass_utils import run_bass_kernel_spmd
from concourse.tile import add_dep_helper

N_CORES = 8
N = 16384          # tokens (128 x 128 grid)
D = 512            # d_model
T = N // N_CORES   # tokens per core
P = 128            # partitions
TT = T // P        # token tiles per core
KT = D // P        # contraction tiles
OB = 2             # token tiles batched per output DMA
OBUFS = 4          # output staging buffers
POB = 4            # main psum bufs
F32 = mybir.dt.float32
F32R = mybir.dt.float32r  # fp32 4-xbus mode: 1 cyc/row when moving dim >= 256

_PROGRAM_CACHE = {}


XCHUNKS = 4        # input DMA split count (sub-range deps let compute start early)


def build_program(with_bias: bool) -> bacc.Bacc:
    nc = bacc.Bacc("TRN2", target_bir_lowering=False, debug=False)
    xt = nc.dram_tensor("xt", [D, T], F32R, kind="ExternalInput").ap()
    wvt = nc.dram_tensor("wvt", [D, D], F32R, kind="ExternalInput").ap()
    wo = nc.dram_tensor("wo", [D, D], F32R, kind="ExternalInput").ap()
    if with_bias:
        bvb = nc.dram_tensor("bvb", [1, D], F32R, kind="ExternalInput").ap()
        bob = nc.dram_tensor("bob", [1, D], F32, kind="ExternalInput").ap()
    out = nc.dram_tensor("out", [T, D], F32, kind="ExternalOutput").ap()

    with tile.TileContext(nc) as tc:
        with (
            tc.tile_pool(name="consts", bufs=1) as consts,
            tc.tile_pool(name="wpool", bufs=1) as wpool,
            tc.tile_pool(name="opool", bufs=OBUFS) as opool,
            tc.tile_pool(name="po", bufs=POB, space="PSUM") as po,
            tc.tile_pool(name="pw", bufs=2, space="PSUM") as pw,
        ):
            # Weights first: the fold gates the main loop, so their DMAs
            # must not queue behind the 4 MB x transfer.
            wvt_sb = wpool.tile([P, KT, D], F32R)
            wo_sb = wpool.tile([P, KT, D], F32R)
            wvt_r = wvt.rearrange("(k p) i -> p k i", p=P)
            wo_r = wo.rearrange("(k p) j -> p k j", p=P)
            for k in range(KT):
                nc.sync.dma_start(out=wvt_sb[:, k:k + 1, :], in_=wvt_r[:, k:k + 1, :])
                nc.sync.dma_start(out=wo_sb[:, k:k + 1, :], in_=wo_r[:, k:k + 1, :])

            # Fold W = Wv @ Wo in fp32r (operands pre-rounded on host, fp32
            # PSUM accumulate); the PSUM->SBUF copy re-rounds W to fp32r.
            w_sb = wpool.tile([P, KT, D], F32R)
            fold_mm0 = None
            for i in range(KT):
                psw = pw.tile([P, D], F32, tag="psw", name=f"psw{i}")
                for k in range(KT):
                    mm = nc.tensor.matmul(
                        psw,
                        lhsT=wvt_sb[:, k, i * P:(i + 1) * P],
                        rhs=wo_sb[:, k, :],
                        start=(k == 0),
                        stop=(k == KT - 1),
                    )
                    if fold_mm0 is None:
                        fold_mm0 = mm
                nc.vector.tensor_copy(out=w_sb[:, i, :], in_=psw)

            # x arrives pre-transposed + pre-rounded: xtr[p, k, t] = x.T rows.
            # Gate the 4 MB transfer on the fold's first matmul so the weight
            # DMAs get the full HBM bandwidth during the critical head.
            xtr = wpool.tile([P, KT, T], F32R)
            xt_r = xt.rearrange("(k p) t -> p k t", p=P)
            cw = T // XCHUNKS
            for m in range(XCHUNKS):
                xdma = nc.sync.dma_start(
                    out=xtr[:, :, m * cw:(m + 1) * cw],
                    in_=xt_r[:, :, m * cw:(m + 1) * cw],
                )
                add_dep_helper(xdma.ins, fold_mm0.ins,
                               reason="x-dma after weights landed")

            if with_bias:
                # beff = bv @ Wo + bo, as a [1, D] row.
                ones = consts.tile([1, P], F32)
                nc.vector.memset(ones, 1.0)
                bv_sb = consts.tile([P, KT], F32R)
                nc.sync.dma_start(
                    out=bv_sb, in_=bvb.rearrange("o (k p) -> p (o k)", p=P)
                )
                bo_sb = consts.tile([1, D], F32)
                nc.sync.dma_start(out=bo_sb, in_=bob)
                psb = pw.tile([1, D], F32, tag="psw", name="psb")
                for k in range(KT):
                    nc.tensor.matmul(
                        psb,
                        lhsT=bv_sb[:, k:k + 1],
                        rhs=wo_sb[:, k, :],
                        start=(k == 0),
                        stop=(k == KT - 1),
                    )
                beff_sb = consts.tile([1, D], F32)
                nc.vector.tensor_tensor(
                    out=beff_sb, in0=psb, in1=bo_sb, op=mybir.AluOpType.add
                )

            # Main loop: 4 accumulating fp32r matmuls per 128-token tile,
            # PSUM->SBUF copies alternating DVE/ACT, 1 MB batched stores.
            for c in range(TT // OB):
                obuf = opool.tile([P, OB, D], F32, tag="ob", name=f"ob{c}")
                for s in range(OB):
                    t = c * OB + s
                    pso = po.tile([P, D], F32, tag="pso", name=f"pso{t}")
                    for k in range(KT):
                        nc.tensor.matmul(
                            pso,
                            lhsT=xtr[:, k, t * P:(t + 1) * P],
                            rhs=w_sb[:, k, :],
                            start=(k == 0),
                            stop=(k == KT - 1 and not with_bias),
                        )
                    if with_bias:
                        nc.tensor.matmul(
                            pso, lhsT=ones, rhs=beff_sb, start=False, stop=True
                        )
                    nc.vector.tensor_copy(out=obuf[:, s, :], in_=pso)
                nc.sync.dma_start(
                    out=out[c * OB * P:(c + 1) * OB * P, :].rearrange(
                        "(s p) d -> p s d", p=P
                    ),
                    in_=obuf,
                )
    nc.compile()  # bacc: legalizes waits (<=1 per inst via event semaphores)
    return nc


def _get_program(with_bias: bool) -> bacc.Bacc:
    if with_bias not in _PROGRAM_CACHE:
        _PROGRAM_CACHE[with_bias] = build_program(with_bias)
    return _PROGRAM_CACHE[with_bias]


def _round_fp32r(a: np.ndarray) -> np.ndarray:
    """Round fp32 values to the fp32r grid (12 explicit mantissa bits)."""
    u = np.ascontiguousarray(a, dtype=np.float32).view(np.uint32)
    u = ((u + np.uint32(0x800)) & np.uint32(0xFFFFF000)).astype(np.uint32)
    return u.view(np.float32)


def make_in_maps(x, Wv, bv, Wo, bo):
    x2 = np.asarray(x, dtype=np.float32).reshape(N, D)
    wvt_np = _round_fp32r(np.asarray(Wv, dtype=np.float32).T)
    wo_np = _round_fp32r(np.asarray(Wo, dtype=np.float32))
    bv_np = _round_fp32r(np.asarray(bv, dtype=np.float32).reshape(1, D))
    bo_np = np.asarray(bo, dtype=np.float32).reshape(1, D)
    with_bias = bool(np.any(bv_np) or np.any(bo_np))
    in_maps = []
    for c in range(N_CORES):
        xt_c = _round_fp32r(x2[c * T:(c + 1) * T].T)  # [D, T], fp32r grid
        m = {"xt": xt_c, "wvt": wvt_np, "wo": wo_np}
        if with_bias:
            m["bvb"] = bv_np
            m["bob"] = bo_np
        in_maps.append(m)
    return in_maps, with_bias


def kernel(x, H, W, Wq, bq, Wk, bk, Wv, bv, Wo, bo, Woff1, boff1, Woff2, boff2,
           **_ignored):
    in_maps, with_bias = make_in_maps(x, Wv, bv, Wo, bo)
    nc = _get_program(with_bias)
    res = run_bass_kernel_spmd(nc, in_maps, core_ids=list(range(N_CORES)))
    full = np.concatenate(
        [res.results[c]["out"] for c in range(N_CORES)], axis=0
    )
    return full.reshape(1, N, D).astype(np.float32, copy=False)
